# revision 3
# baseline (speedup 1.0000x reference)
"""Trainium2 Bass kernel for BayesLinearEMP (moe_routing).

out[b] = weights[mode_idx[b]] @ x[b] + biases[mode_idx[b]]
  x: [128, 2048] f32, weights: [20, 2048, 2048] f32, biases: [20, 2048] f32,
  mode_idx: [128] int

Strategy (8 NeuronCores):
  - Split the output dim O=2048 into 8 slices of 256, one per core.  Every
    core sees all 20 modes' weights for its O-slice: 42 MB/core, perfectly
    balanced regardless of the mode distribution (total weight traffic is
    read-once = 335 MB, the memory-roofline minimum).
  - On the host, sort samples by mode.  Per mode m with count c_m the core
    computes a [c_m, 256] tile as 16 K-chunk matmuls (K=128, N=256).
  - fp32 matmuls run at 1/4 PE rate, so each fp32 operand is split into a
    bf16 hi/lo pair ("pseudo-fp32"): W@x = Whi@xhi + Whi@xlo + Wlo@xhi
    (+ Wlo@xlo, dropped: ~2^-18 relative).  3 bf16 passes ≈ rel err ~1e-5
    at full PE rate; DMA traffic is unchanged vs fp32 (2x2B planes).
  - The bias is folded into the PSUM accumulation with a K=2 ones-matmul
    against the [bias_hi; bias_lo] planes.
"""

import sys

for _p in ("/opt/trn_rl_repo", "/root/.axon_site/_ro/trn_rl_repo"):
    if _p not in sys.path:
        sys.path.append(_p)

import numpy as np
import ml_dtypes

BF16 = ml_dtypes.bfloat16

B, I, O, M = 128, 2048, 2048, 20
NCORES = 8
OC = O // NCORES          # 256 output cols per core
KC = I // 128             # 16 contraction chunks

_CACHE: dict = {}
LAST_EXEC_TIME_NS = None


def _install_ntff_shim():
    """antenv.axon_hooks is absent in this image; recreate it so the
    trace=True path of run_bass_kernel_spmd can reach NTFF profiling."""
    import types
    import antenv

    if getattr(antenv, "axon_hooks", None) is not None:
        return
    hooks_mod = types.ModuleType("antenv.axon_hooks")
    _hook = [None]
    hooks_mod.set_axon_ntff_profile_hook = lambda h: _hook.__setitem__(0, h)
    hooks_mod.get_axon_ntff_profile_hook = lambda: _hook[0]
    sys.modules["antenv.axon_hooks"] = hooks_mod
    antenv.axon_hooks = hooks_mod
    try:
        from trn_agent_boot.trn_boot import _ntff_profile_via_ctypes

        hooks_mod.set_axon_ntff_profile_hook(
            _ntff_profile_via_ctypes("/opt/axon/libaxon_pjrt.so")
        )
    except Exception:
        pass
    import concourse.bass_utils as bass_utils

    bass_utils.upload_artifacts = lambda tmpdir: "local://" + tmpdir


def _build(counts: tuple) -> "bacc.Bacc":
    import concourse.bass as bass
    import concourse.tile as tile
    from concourse import bacc, mybir

    offs = np.concatenate([[0], np.cumsum(counts)]).astype(int)

    nc = bacc.Bacc("TRN2", target_bir_lowering=False, debug=False, num_devices=NCORES)
    bf = mybir.dt.bfloat16
    f32 = mybir.dt.float32

    wh_d = nc.dram_tensor("wh", [M, 2, 128, KC * OC], bf, kind="ExternalInput").ap()
    xt_d = nc.dram_tensor("xt", [2, 128, KC * 128], bf, kind="ExternalInput").ap()
    bh_d = nc.dram_tensor("bh", [2, M * OC], bf, kind="ExternalInput").ap()
    out_d = nc.dram_tensor("out", [B, OC], f32, kind="ExternalOutput").ap()

    with tile.TileContext(nc) as tc:
        with (
            tc.tile_pool(name="w", bufs=6) as wpool,
            tc.tile_pool(name="x", bufs=1) as xpool,
            tc.tile_pool(name="consts", bufs=1) as cpool,
            tc.tile_pool(name="o", bufs=3) as opool,
            tc.tile_pool(name="ps", bufs=4, space=bass.MemorySpace.PSUM) as pspool,
        ):
            # critical-path-first: x hi plane, then mode 0's hi weights land
            # before anything else so the PE can start ~10us earlier.
            xhi_t = xpool.tile([128, KC * 128], bf, tag="xhi")
            nc.sync.dma_start(xhi_t[:], xt_d[0])
            xlo_t = xpool.tile([128, KC * 128], bf, tag="xlo")
            nc.sync.dma_start(xlo_t[:], xt_d[1])
            xts = (xhi_t, xlo_t)
            bt = cpool.tile([2, M * OC], bf)
            nc.sync.dma_start(bt[:], bh_d[:])
            ones = cpool.tile([2, 128], bf)
            nc.vector.memset(ones[:], 1.0)

            # (wplane, xplane) terms of (Whi+Wlo) @ (xhi+xlo), Wlo@xlo dropped.
            # Grouped by wplane so T1/T2 only need the hi weight tile.
            combos = ((0, 0), (0, 1), (1, 0))

            for m in range(M):
                cm = int(counts[m])
                if cm == 0:
                    continue
                o0 = int(offs[m])
                wa = wpool.tile([128, KC * OC], bf, tag="w")
                nc.sync.dma_start(wa[:], wh_d[m, 0])
                wb = wpool.tile([128, KC * OC], bf, tag="w")
                nc.sync.dma_start(wb[:], wh_d[m, 1])
                wts = (wa, wb)
                ps = pspool.tile([128, OC], f32)
                first = True
                for tw, tx in combos:
                    for k in range(KC):
                        xoff = k * 128 + o0
                        woff = k * OC
                        nc.tensor.matmul(
                            ps[0:cm, :],
                            xts[tx][:, xoff : xoff + cm],
                            wts[tw][:, woff : woff + OC],
                            start=first,
                            stop=False,
                        )
                        first = False
                nc.tensor.matmul(
                    ps[0:cm, :],
                    ones[:, 0:cm],
                    bt[:, m * OC : (m + 1) * OC],
                    start=False,
                    stop=True,
                )
                ot = opool.tile([128, OC], f32)
                nc.vector.tensor_copy(ot[0:cm, :], ps[0:cm, :])
                nc.scalar.dma_start(out_d[o0 : o0 + cm, :], ot[0:cm, :])

    nc.compile()
    return nc


def _hi_lo(a: np.ndarray):
    hi = a.astype(BF16)
    lo = (a - hi.astype(np.float32)).astype(BF16)
    return hi, lo


def kernel(x, weights, biases, mode_idx):
    global LAST_EXEC_TIME_NS
    import os

    x = np.asarray(x, dtype=np.float32)
    weights = np.asarray(weights, dtype=np.float32)
    biases = np.asarray(biases, dtype=np.float32)
    mode_idx_np = np.asarray(mode_idx).astype(np.int64)

    assert x.shape == (B, I) and weights.shape == (M, O, I)
    assert biases.shape == (M, O) and mode_idx_np.shape == (B,)

    order = np.argsort(mode_idx_np, kind="stable")
    counts = np.bincount(mode_idx_np, minlength=M)
    key = tuple(int(c) for c in counts)

    if key not in _CACHE:
        _CACHE[key] = _build(key)
    nc = _CACHE[key]

    # --- host-side data prep into the on-chip layouts ---
    xs = x[order]                                    # [B, I] sorted by mode
    xhi, xlo = _hi_lo(xs)
    xpl = np.stack([xhi, xlo], 0)                    # [t, s, i]
    XT = np.ascontiguousarray(
        xpl.reshape(2, B, KC, 128).transpose(0, 3, 2, 1)   # [t, p, k, s]
    ).reshape(2, 128, KC * 128)

    whi, wlo = _hi_lo(weights)
    wpl = np.stack([whi, wlo], 0)                    # [t, m, o, i]
    WH = np.ascontiguousarray(
        wpl.reshape(2, M, NCORES, OC, KC, 128).transpose(2, 1, 0, 5, 4, 3)
    ).reshape(NCORES, M, 2, 128, KC * OC)            # [c, m, t, p, (k,cc)]

    bhi, blo = _hi_lo(biases)
    bpl = np.stack([bhi, blo], 0)                    # [t, m, o]
    BH = np.ascontiguousarray(
        bpl.reshape(2, M, NCORES, OC).transpose(2, 0, 1, 3)
    ).reshape(NCORES, 2, M * OC)

    in_maps = [{"wh": WH[c], "xt": XT, "bh": BH[c]} for c in range(NCORES)]

    from concourse.bass_utils import run_bass_kernel_spmd

    trace = bool(int(os.environ.get("BASS_KERNEL_TRACE", "0")))
    if trace:
        _install_ntff_shim()
    res = run_bass_kernel_spmd(
        nc,
        in_maps,
        list(range(NCORES)),
        trace=trace,
        trace_cores=list(range(NCORES)) if trace else None,
    )
    LAST_EXEC_TIME_NS = res.exec_time_ns

    sorted_out = np.concatenate(
        [res.results[c]["out"] for c in range(NCORES)], axis=1
    )                                                # [B, O] in sorted order
    out = np.empty((B, O), dtype=np.float32)
    out[order] = sorted_out
    return out


# revision 7
# speedup vs baseline: 1.1691x; 1.1691x over previous
"""Trainium2 Bass kernel for BayesLinearEMP (moe_routing).

out[b] = weights[mode_idx[b]] @ x[b] + biases[mode_idx[b]]
  x: [128, 2048] f32, weights: [20, 2048, 2048] f32, biases: [20, 2048] f32,
  mode_idx: [128] int

Strategy (8 NeuronCores):
  - Split the output dim O=2048 into 8 slices of 256, one per core.  Every
    core reads all 20 modes' weights for its O-slice — perfectly balanced
    regardless of the mode distribution, and total weight traffic is
    read-once (the memory-roofline minimum).
  - On the host, sort samples by mode.  Per mode m with count c_m the core
    computes a [c_m, 256] tile as 16 K-chunk matmuls (K=128, N=256),
    accumulated in PSUM; per-mode counts are compile-time constants
    (program cached per counts-tuple).
  - fp32 matmuls run at 1/4 PE rate, so fp32 operands are split into
    multi-plane low-precision pairs at full PE rate.  Default "f16f8":
      W*64 = W1(fp16) + R;  W2 = fp8e4m3(R*512)      (21 + 10.5 MB/core)
      x = x1(fp16) + x2;  x2s = fp16(x2*512);  x3 = fp8e4m3(x)
      ps_main = W1@x1 + 64*bias   (fp16 matmuls + bf16 bias-ones matmul)
      ps23    = W1@x2s + W2@x3    (= 2^15 * (W@x2 + W2@x))
      out*64  = ps_main + 2^-9 * ps23   (DVE), host divides by 64.
    All stored plane values sit in each format's normal range, so the
    result is exact to ~2^-15 regardless of PE subnormal handling
    (measured rel err ~1e-5).  Mode "bf16x2" (42 MB/core, ~4e-6) kept as
    fallback via ACCURACY_MODE=bf16x2.
  - The bias is folded into the PSUM accumulation with a K=2 ones-matmul
    against the [bias_hi; bias_lo] bf16 planes.
"""

import os
import sys

for _p in ("/opt/trn_rl_repo", "/root/.axon_site/_ro/trn_rl_repo"):
    if _p not in sys.path:
        sys.path.append(_p)

import numpy as np
import ml_dtypes

BF16 = ml_dtypes.bfloat16
F16 = np.float16
F8 = ml_dtypes.float8_e4m3

B, I, O, M = 128, 2048, 2048, 20
NCORES = 8
OC = O // NCORES          # 256 output cols per core
KC = I // 128             # 16 contraction chunks

MODE = os.environ.get("ACCURACY_MODE", "f16f8")

_CACHE: dict = {}
LAST_EXEC_TIME_NS = None


def _install_ntff_shim():
    """antenv.axon_hooks is absent in this image; recreate it so the
    trace=True path of run_bass_kernel_spmd can reach NTFF profiling."""
    import types
    import antenv

    if getattr(antenv, "axon_hooks", None) is not None:
        return
    hooks_mod = types.ModuleType("antenv.axon_hooks")
    _hook = [None]
    hooks_mod.set_axon_ntff_profile_hook = lambda h: _hook.__setitem__(0, h)
    hooks_mod.get_axon_ntff_profile_hook = lambda: _hook[0]
    sys.modules["antenv.axon_hooks"] = hooks_mod
    antenv.axon_hooks = hooks_mod
    try:
        from trn_agent_boot.trn_boot import _ntff_profile_via_ctypes

        hooks_mod.set_axon_ntff_profile_hook(
            _ntff_profile_via_ctypes("/opt/axon/libaxon_pjrt.so")
        )
    except Exception:
        pass
    import concourse.bass_utils as bass_utils

    bass_utils.upload_artifacts = lambda tmpdir: "local://" + tmpdir


def _build(counts: tuple, mode: str):
    import concourse.bass as bass
    import concourse.tile as tile
    from concourse import bacc, mybir

    offs = np.concatenate([[0], np.cumsum(counts)]).astype(int)

    nc = bacc.Bacc("TRN2", target_bir_lowering=False, debug=False, num_devices=NCORES)
    bf = mybir.dt.bfloat16
    f16 = mybir.dt.float16
    f8 = mybir.dt.float8e4
    f32 = mybir.dt.float32

    if mode == "f16f8":
        dt_a, dt_b, dt_x12 = f16, f8, f16
        comb = 2.0 ** -9
    else:  # bf16x2: T3 = Wlo @ xhi at scale 1
        dt_a, dt_b, dt_x12 = bf, bf, bf
        comb = 1.0

    wa_d = nc.dram_tensor("wa", [M, 128, KC * OC], dt_a, kind="ExternalInput").ap()
    wb_d = nc.dram_tensor("wb", [M, 128, KC * OC], dt_b, kind="ExternalInput").ap()
    x1_d = nc.dram_tensor("x1", [128, KC * 128], dt_x12, kind="ExternalInput").ap()
    x2_d = nc.dram_tensor("x2", [128, KC * 128], dt_x12, kind="ExternalInput").ap()
    x3_d = nc.dram_tensor("x3", [128, KC * 128], dt_b, kind="ExternalInput").ap()
    bh_d = nc.dram_tensor("bh", [2, M * OC], bf, kind="ExternalInput").ap()
    out_d = nc.dram_tensor("out", [B, OC], f32, kind="ExternalOutput").ap()

    with tile.TileContext(nc) as tc:
        with (
            tc.tile_pool(name="w", bufs=4) as wpool,
            tc.tile_pool(name="x", bufs=1) as xpool,
            tc.tile_pool(name="consts", bufs=1) as cpool,
            tc.tile_pool(name="o", bufs=3) as opool,
            tc.tile_pool(name="ps", bufs=3, space=bass.MemorySpace.PSUM) as pspool,
        ):
            # critical-path-first: x1 + mode 0's main weights unblock the PE
            x1t = xpool.tile([128, KC * 128], dt_x12, tag="x1")
            nc.sync.dma_start(x1t[:], x1_d[:])
            x2t = xpool.tile([128, KC * 128], dt_x12, tag="x2")
            nc.sync.dma_start(x2t[:], x2_d[:])
            x3t = xpool.tile([128, KC * 128], dt_b, tag="x3")
            nc.sync.dma_start(x3t[:], x3_d[:])
            bt = cpool.tile([2, M * OC], bf)
            nc.sync.dma_start(bt[:], bh_d[:])
            ones = cpool.tile([2, 128], bf)
            nc.vector.memset(ones[:], 1.0)

            for m in range(M):
                cm = int(counts[m])
                if cm == 0:
                    continue
                o0 = int(offs[m])
                wa = wpool.tile([128, KC * OC], dt_a, tag="wa")
                nc.sync.dma_start(wa[:], wa_d[m])
                wb = wpool.tile([128, KC * OC], dt_b, tag="wb")
                nc.sync.dma_start(wb[:], wb_d[m])

                ps_main = pspool.tile([128, OC], f32, tag="ps_main")
                ps23 = pspool.tile([128, OC], f32, tag="ps23")

                # T1: W1 @ x1 -> ps_main
                for k in range(KC):
                    nc.tensor.matmul(
                        ps_main[0:cm, :],
                        x1t[:, k * 128 + o0 : k * 128 + o0 + cm],
                        wa[:, k * OC : (k + 1) * OC],
                        start=(k == 0),
                        stop=False,
                    )
                # bias (scaled by 64 on host): ones[2,cm].T @ [bh; bl]
                nc.tensor.matmul(
                    ps_main[0:cm, :],
                    ones[:, 0:cm],
                    bt[:, m * OC : (m + 1) * OC],
                    start=False,
                    stop=True,
                )
                # T2: W1 @ x2s, T3: W2 @ x3 -> ps23 (both at 2^15 scale)
                for k in range(KC):
                    nc.tensor.matmul(
                        ps23[0:cm, :],
                        x2t[:, k * 128 + o0 : k * 128 + o0 + cm],
                        wa[:, k * OC : (k + 1) * OC],
                        start=(k == 0),
                        stop=False,
                    )
                for k in range(KC):
                    nc.tensor.matmul(
                        ps23[0:cm, :],
                        x3t[:, k * 128 + o0 : k * 128 + o0 + cm],
                        wb[:, k * OC : (k + 1) * OC],
                        start=False,
                        stop=(k == KC - 1),
                    )

                tmp = opool.tile([128, OC], f32, tag="tmp")
                nc.vector.tensor_scalar_mul(tmp[0:cm, :], ps23[0:cm, :], comb)
                ot = opool.tile([128, OC], f32, tag="ot")
                nc.vector.tensor_add(ot[0:cm, :], ps_main[0:cm, :], tmp[0:cm, :])
                nc.scalar.dma_start(out_d[o0 : o0 + cm, :], ot[0:cm, :])

    nc.compile()
    return nc


def _w_layout(plane: np.ndarray, dt) -> np.ndarray:
    """[m, o, i] -> [core, m, p, k*OC] so each (mode, core) DMA is one
    contiguous-per-partition [128, KC*OC] tile with rhs chunks in order."""
    return np.ascontiguousarray(
        plane.reshape(M, NCORES, OC, KC, 128).transpose(1, 0, 4, 3, 2).astype(dt)
    ).reshape(NCORES, M, 128, KC * OC)


def _x_layout(plane: np.ndarray, dt) -> np.ndarray:
    """[s, i] -> [p, k*128] (lhsT chunks: partition = i within chunk)."""
    return np.ascontiguousarray(
        plane.reshape(B, KC, 128).transpose(2, 1, 0).astype(dt)
    ).reshape(128, KC * 128)


def kernel(x, weights, biases, mode_idx):
    global LAST_EXEC_TIME_NS

    x = np.asarray(x, dtype=np.float32)
    weights = np.asarray(weights, dtype=np.float32)
    biases = np.asarray(biases, dtype=np.float32)
    mode_idx_np = np.asarray(mode_idx).astype(np.int64)

    assert x.shape == (B, I) and weights.shape == (M, O, I)
    assert biases.shape == (M, O) and mode_idx_np.shape == (B,)

    order = np.argsort(mode_idx_np, kind="stable")
    counts = np.bincount(mode_idx_np, minlength=M)
    key = (tuple(int(c) for c in counts), MODE)

    if key not in _CACHE:
        _CACHE[key] = _build(key[0], MODE)
    nc = _CACHE[key]

    xs = x[order]                                    # [B, I] sorted by mode

    if MODE == "f16f8":
        ws = weights * np.float32(64.0)
        w1 = ws.astype(F16)
        r = ws - w1.astype(np.float32)
        WA = _w_layout(w1, F16)
        WB = _w_layout(r * np.float32(512.0), F8)
        del ws, r

        x1 = xs.astype(F16)
        x2 = (xs - x1.astype(np.float32)) * np.float32(512.0)
        X1 = _x_layout(x1, F16)
        X2 = _x_layout(x2, F16)
        X3 = _x_layout(xs, F8)

        bs = biases * np.float32(64.0)
        out_scale = np.float32(1.0 / 64.0)
    else:  # bf16x2
        w1 = weights.astype(BF16)
        r = weights - w1.astype(np.float32)
        WA = _w_layout(w1, BF16)
        WB = _w_layout(r, BF16)

        x1 = xs.astype(BF16)
        x2 = xs - x1.astype(np.float32)
        X1 = _x_layout(x1, BF16)
        X2 = _x_layout(x2, BF16)
        X3 = X1                                      # T3 = Wlo @ xhi

        bs = biases
        out_scale = np.float32(1.0)

    bh = bs.astype(BF16)
    bl = (bs - bh.astype(np.float32)).astype(BF16)
    bpl = np.stack([bh, bl], 0)                      # [t, m, o]
    BH = np.ascontiguousarray(
        bpl.reshape(2, M, NCORES, OC).transpose(2, 0, 1, 3)
    ).reshape(NCORES, 2, M * OC)

    in_maps = [
        {"wa": WA[c], "wb": WB[c], "x1": X1, "x2": X2, "x3": X3, "bh": BH[c]}
        for c in range(NCORES)
    ]

    from concourse.bass_utils import run_bass_kernel_spmd

    trace = bool(int(os.environ.get("BASS_KERNEL_TRACE", "0")))
    if trace:
        _install_ntff_shim()
    res = run_bass_kernel_spmd(
        nc,
        in_maps,
        list(range(NCORES)),
        trace=trace,
        trace_cores=list(range(NCORES)) if trace else None,
    )
    LAST_EXEC_TIME_NS = res.exec_time_ns

    sorted_out = np.concatenate(
        [res.results[c]["out"] for c in range(NCORES)], axis=1
    )                                                # [B, O] in sorted order
    out = np.empty((B, O), dtype=np.float32)
    out[order] = sorted_out * out_scale
    return out


# revision 13
# speedup vs baseline: 1.2786x; 1.0937x over previous
"""Trainium2 Bass kernel for BayesLinearEMP (moe_routing).

out[b] = weights[mode_idx[b]] @ x[b] + biases[mode_idx[b]]
  x: [128, 2048] f32, weights: [20, 2048, 2048] f32, biases: [20, 2048] f32,
  mode_idx: [128] int

Strategy (8 NeuronCores):
  - Split the output dim O=2048 into 8 slices of 256, one per core.  Every
    core reads all 20 modes' weights for its O-slice — perfectly balanced
    regardless of the mode distribution, and total weight traffic is
    read-once (the memory-roofline minimum).
  - On the host, sort samples by mode.  Per mode m with count c_m the core
    computes a [c_m, 256] tile as 16 K-chunk matmuls (K=128, N=256),
    accumulated in PSUM; per-mode counts are compile-time constants
    (program cached per counts-tuple).
  - fp32 matmuls run at 1/4 PE rate, so fp32 operands are split into
    multi-plane low-precision pairs at full PE rate.  Default "f16f8":
      W*64 = W1(fp16) + R;  W2 = fp8e4m3(R*512)      (21 + 10.5 MB/core)
      x = x1(fp16) + x2;  x2s = fp16(x2*512);  x3 = fp8e4m3(x)
      ps_main = W1@x1 + 64*bias   (fp16 matmuls + bf16 bias-ones matmul)
      ps23    = W1@x2s + W2@x3    (= 2^15 * (W@x2 + W2@x))
      out*64  = ps_main + 2^-9 * ps23   (DVE), host divides by 64.
    All stored plane values sit in each format's normal range, so the
    result is exact to ~2^-15 regardless of PE subnormal handling
    (measured rel err ~1e-5).  Mode "bf16x2" (42 MB/core, ~4e-6) kept as
    fallback via ACCURACY_MODE=bf16x2.
  - The bias is folded into the PSUM accumulation with a K=2 ones-matmul
    against the [bias_hi; bias_lo] bf16 planes.
"""

import os
import sys

for _p in ("/opt/trn_rl_repo", "/root/.axon_site/_ro/trn_rl_repo"):
    if _p not in sys.path:
        sys.path.append(_p)

import numpy as np
import ml_dtypes

BF16 = ml_dtypes.bfloat16
F16 = np.float16
F8 = ml_dtypes.float8_e4m3

B, I, O, M = 128, 2048, 2048, 20
NCORES = 8
OC = O // NCORES          # 256 output cols per core
KC = I // 128             # 16 contraction chunks

MODE = os.environ.get("ACCURACY_MODE", "f16f8")

_CACHE: dict = {}
LAST_EXEC_TIME_NS = None


def _install_ntff_shim():
    """antenv.axon_hooks is absent in this image; recreate it so the
    trace=True path of run_bass_kernel_spmd can reach NTFF profiling."""
    import types
    import antenv

    if getattr(antenv, "axon_hooks", None) is not None:
        return
    hooks_mod = types.ModuleType("antenv.axon_hooks")
    _hook = [None]
    hooks_mod.set_axon_ntff_profile_hook = lambda h: _hook.__setitem__(0, h)
    hooks_mod.get_axon_ntff_profile_hook = lambda: _hook[0]
    sys.modules["antenv.axon_hooks"] = hooks_mod
    antenv.axon_hooks = hooks_mod
    try:
        from trn_agent_boot.trn_boot import _ntff_profile_via_ctypes

        hooks_mod.set_axon_ntff_profile_hook(
            _ntff_profile_via_ctypes("/opt/axon/libaxon_pjrt.so")
        )
    except Exception:
        pass
    import concourse.bass_utils as bass_utils

    bass_utils.upload_artifacts = lambda tmpdir: "local://" + tmpdir


def _build(counts: tuple, mode: str):
    import concourse.bass as bass
    import concourse.tile as tile
    from concourse import bacc, mybir

    offs = np.concatenate([[0], np.cumsum(counts)]).astype(int)

    nc = bacc.Bacc("TRN2", target_bir_lowering=False, debug=False, num_devices=NCORES)
    bf = mybir.dt.bfloat16
    f16 = mybir.dt.float16
    f8 = mybir.dt.float8e4
    f32 = mybir.dt.float32

    if mode == "f16f8":
        dt_a, dt_b, dt_x12 = f16, f8, f16
        comb = 2.0 ** -9
    else:  # bf16x2: T3 = Wlo @ xhi at scale 1
        dt_a, dt_b, dt_x12 = bf, bf, bf
        comb = 1.0

    # f16f8: T3 runs as fp8 DoubleRow (2 fp8 weights/PE cell, 0.5 cyc/row):
    # contraction chunks of 256 as [p, pair] with i = k'*256 + pair*128 + p.
    dr = mode == "f16f8"
    KD = KC // 2

    wa_d = nc.dram_tensor("wa", [M, 128, KC * OC], dt_a, kind="ExternalInput").ap()
    if dr:
        wb_d = nc.dram_tensor("wb", [M, 128, KD, 2, OC], dt_b, kind="ExternalInput").ap()
        x3_d = nc.dram_tensor("x3", [128, KD, 2, 128], dt_b, kind="ExternalInput").ap()
    else:
        wb_d = nc.dram_tensor("wb", [M, 128, KC * OC], dt_b, kind="ExternalInput").ap()
        x3_d = nc.dram_tensor("x3", [128, KC * 128], dt_b, kind="ExternalInput").ap()
    x1_d = nc.dram_tensor("x1", [128, KC * 128], dt_x12, kind="ExternalInput").ap()
    x2_d = nc.dram_tensor("x2", [128, KC * 128], dt_x12, kind="ExternalInput").ap()
    bh_d = nc.dram_tensor("bh", [2, M * OC], bf, kind="ExternalInput").ap()
    out_d = nc.dram_tensor("out", [B, OC], f32, kind="ExternalOutput").ap()

    with tile.TileContext(nc) as tc:
        with (
            tc.tile_pool(name="w", bufs=4) as wpool,
            tc.tile_pool(name="x", bufs=1) as xpool,
            tc.tile_pool(name="consts", bufs=1) as cpool,
            tc.tile_pool(name="o", bufs=3) as opool,
            tc.tile_pool(name="ps", bufs=3, space=bass.MemorySpace.PSUM) as pspool,
        ):
            # critical-path-first: x1 + mode 0's main weights go on the sync
            # ring; everything else rides the scalar HWDGE ring in parallel.
            x1t = xpool.tile([128, KC * 128], dt_x12, tag="x1")
            nc.sync.dma_start(x1t[:], x1_d[:])
            x2t = xpool.tile([128, KC * 128], dt_x12, tag="x2")
            nc.scalar.dma_start(x2t[:], x2_d[:])
            if dr:
                x3t = xpool.tile([128, KD, 2, 128], dt_b, tag="x3")
            else:
                x3t = xpool.tile([128, KC * 128], dt_b, tag="x3")
            nc.scalar.dma_start(x3t[:], x3_d[:])
            bt = cpool.tile([2, M * OC], bf)
            nc.scalar.dma_start(bt[:], bh_d[:])
            ones = cpool.tile([2, 128], bf)
            nc.vector.memset(ones[:], 1.0)

            for m in range(M):
                cm = int(counts[m])
                if cm == 0:
                    continue
                o0 = int(offs[m])
                wa = wpool.tile([128, KC * OC], dt_a, tag="wa")
                nc.sync.dma_start(wa[:], wa_d[m])
                if dr:
                    wb = wpool.tile([128, KD, 2, OC], dt_b, tag="wb")
                else:
                    wb = wpool.tile([128, KC * OC], dt_b, tag="wb")
                nc.sync.dma_start(wb[:], wb_d[m])

                ps_main = pspool.tile([128, OC], f32, tag="ps_main")
                ps23 = pspool.tile([128, OC], f32, tag="ps23")

                # T1: W1 @ x1 -> ps_main
                for k in range(KC):
                    nc.tensor.matmul(
                        ps_main[0:cm, :],
                        x1t[:, k * 128 + o0 : k * 128 + o0 + cm],
                        wa[:, k * OC : (k + 1) * OC],
                        start=(k == 0),
                        stop=False,
                    )
                # bias (scaled by 64 on host): ones[2,cm].T @ [bh; bl]
                nc.tensor.matmul(
                    ps_main[0:cm, :],
                    ones[:, 0:cm],
                    bt[:, m * OC : (m + 1) * OC],
                    start=False,
                    stop=True,
                )
                # T2: W1 @ x2s, T3: W2 @ x3 -> ps23 (both at 2^15 scale)
                for k in range(KC):
                    nc.tensor.matmul(
                        ps23[0:cm, :],
                        x2t[:, k * 128 + o0 : k * 128 + o0 + cm],
                        wa[:, k * OC : (k + 1) * OC],
                        start=(k == 0),
                        stop=False,
                    )
                if dr:
                    for kp in range(KD):
                        nc.tensor.matmul(
                            ps23[0:cm, :],
                            x3t[:, kp, :, o0 : o0 + cm],
                            wb[:, kp, :, :],
                            start=False,
                            stop=(kp == KD - 1),
                            perf_mode=mybir.MatmulPerfMode.DoubleRow,
                        )
                else:
                    for k in range(KC):
                        nc.tensor.matmul(
                            ps23[0:cm, :],
                            x3t[:, k * 128 + o0 : k * 128 + o0 + cm],
                            wb[:, k * OC : (k + 1) * OC],
                            start=False,
                            stop=(k == KC - 1),
                        )

                tmp = opool.tile([128, OC], f32, tag="tmp")
                nc.vector.tensor_scalar_mul(tmp[0:cm, :], ps23[0:cm, :], comb)
                ot = opool.tile([128, OC], f32, tag="ot")
                nc.vector.tensor_add(ot[0:cm, :], ps_main[0:cm, :], tmp[0:cm, :])
                nc.scalar.dma_start(out_d[o0 : o0 + cm, :], ot[0:cm, :])

    nc.compile()
    return nc


def _w_layout(plane: np.ndarray, dt) -> np.ndarray:
    """[m, o, i] -> [core, m, p, k*OC] so each (mode, core) DMA is one
    contiguous-per-partition [128, KC*OC] tile with rhs chunks in order."""
    return np.ascontiguousarray(
        plane.reshape(M, NCORES, OC, KC, 128).transpose(1, 0, 4, 3, 2).astype(dt)
    ).reshape(NCORES, M, 128, KC * OC)


def _x_layout(plane: np.ndarray, dt) -> np.ndarray:
    """[s, i] -> [p, k*128] (lhsT chunks: partition = i within chunk)."""
    return np.ascontiguousarray(
        plane.reshape(B, KC, 128).transpose(2, 1, 0).astype(dt)
    ).reshape(128, KC * 128)


def _w_layout_dr(plane: np.ndarray, dt) -> np.ndarray:
    """[m, o, i] -> [core, m, p, k', pair, cc] for fp8 DoubleRow rhs tiles
    (i = k'*256 + pair*128 + p)."""
    return np.ascontiguousarray(
        plane.reshape(M, NCORES, OC, KC // 2, 2, 128)
        .transpose(1, 0, 5, 3, 4, 2)
        .astype(dt)
    )


def _x_layout_dr(plane: np.ndarray, dt) -> np.ndarray:
    """[s, i] -> [p, k', pair, s] for fp8 DoubleRow lhsT tiles."""
    return np.ascontiguousarray(
        plane.reshape(B, KC // 2, 2, 128).transpose(3, 1, 2, 0).astype(dt)
    )


def kernel(x, weights, biases, mode_idx):
    global LAST_EXEC_TIME_NS

    x = np.asarray(x, dtype=np.float32)
    weights = np.asarray(weights, dtype=np.float32)
    biases = np.asarray(biases, dtype=np.float32)
    mode_idx_np = np.asarray(mode_idx).astype(np.int64)

    assert x.shape == (B, I) and weights.shape == (M, O, I)
    assert biases.shape == (M, O) and mode_idx_np.shape == (B,)

    order = np.argsort(mode_idx_np, kind="stable")
    counts = np.bincount(mode_idx_np, minlength=M)
    key = (tuple(int(c) for c in counts), MODE)

    if key not in _CACHE:
        _CACHE[key] = _build(key[0], MODE)
    nc = _CACHE[key]

    xs = x[order]                                    # [B, I] sorted by mode

    if MODE == "f16f8":
        ws = weights * np.float32(64.0)
        w1 = ws.astype(F16)
        r = ws - w1.astype(np.float32)
        WA = _w_layout(w1, F16)
        WB = _w_layout_dr(r * np.float32(512.0), F8)
        del ws, r

        x1 = xs.astype(F16)
        x2 = (xs - x1.astype(np.float32)) * np.float32(512.0)
        X1 = _x_layout(x1, F16)
        X2 = _x_layout(x2, F16)
        X3 = _x_layout_dr(xs, F8)

        bs = biases * np.float32(64.0)
        out_scale = np.float32(1.0 / 64.0)
    else:  # bf16x2
        w1 = weights.astype(BF16)
        r = weights - w1.astype(np.float32)
        WA = _w_layout(w1, BF16)
        WB = _w_layout(r, BF16)

        x1 = xs.astype(BF16)
        x2 = xs - x1.astype(np.float32)
        X1 = _x_layout(x1, BF16)
        X2 = _x_layout(x2, BF16)
        X3 = X1                                      # T3 = Wlo @ xhi

        bs = biases
        out_scale = np.float32(1.0)

    bh = bs.astype(BF16)
    bl = (bs - bh.astype(np.float32)).astype(BF16)
    bpl = np.stack([bh, bl], 0)                      # [t, m, o]
    BH = np.ascontiguousarray(
        bpl.reshape(2, M, NCORES, OC).transpose(2, 0, 1, 3)
    ).reshape(NCORES, 2, M * OC)

    in_maps = [
        {"wa": WA[c], "wb": WB[c], "x1": X1, "x2": X2, "x3": X3, "bh": BH[c]}
        for c in range(NCORES)
    ]

    from concourse.bass_utils import run_bass_kernel_spmd

    trace = bool(int(os.environ.get("BASS_KERNEL_TRACE", "0")))
    if trace:
        _install_ntff_shim()
    res = run_bass_kernel_spmd(
        nc,
        in_maps,
        list(range(NCORES)),
        trace=trace,
        trace_cores=list(range(NCORES)) if trace else None,
    )
    LAST_EXEC_TIME_NS = res.exec_time_ns

    sorted_out = np.concatenate(
        [res.results[c]["out"] for c in range(NCORES)], axis=1
    )                                                # [B, O] in sorted order
    out = np.empty((B, O), dtype=np.float32)
    out[order] = sorted_out * out_scale
    return out


# revision 16
# speedup vs baseline: 1.3234x; 1.0351x over previous
"""Trainium2 Bass kernel for BayesLinearEMP (moe_routing).

out[b] = weights[mode_idx[b]] @ x[b] + biases[mode_idx[b]]
  x: [128, 2048] f32, weights: [20, 2048, 2048] f32, biases: [20, 2048] f32,
  mode_idx: [128] int

Strategy (8 NeuronCores):
  - Split the output dim O=2048 into 8 slices of 256, one per core.  Every
    core reads all 20 modes' weights for its O-slice — perfectly balanced
    regardless of the mode distribution, and total weight traffic is
    read-once (the memory-roofline minimum).
  - On the host, sort samples by mode.  Per mode m with count c_m the core
    computes a [c_m, 256] tile as 16 K-chunk matmuls (K=128, N=256),
    accumulated in PSUM; per-mode counts are compile-time constants
    (program cached per counts-tuple).
  - fp32 matmuls run at 1/4 PE rate, so fp32 operands are split into
    multi-plane low-precision pairs at full PE rate.  Default "f16f8":
      W*64 = W1(fp16) + R;  W2 = fp8e4m3(R*512)      (21 + 10.5 MB/core)
      x = x1(fp16) + x2;  x2s = fp16(x2*512);  x3 = fp8e4m3(x)
      ps_main = W1@x1 + 64*bias   (fp16 matmuls + bf16 bias-ones matmul)
      ps23    = W1@x2s + W2@x3    (= 2^15 * (W@x2 + W2@x))
      out*64  = ps_main + 2^-9 * ps23   (DVE), host divides by 64.
    All stored plane values sit in each format's normal range, so the
    result is exact to ~2^-15 regardless of PE subnormal handling
    (measured rel err ~1e-5).  Mode "bf16x2" (42 MB/core, ~4e-6) kept as
    fallback via ACCURACY_MODE=bf16x2.
  - The bias is folded into the PSUM accumulation with a K=2 ones-matmul
    against the [bias_hi; bias_lo] bf16 planes.
"""

import os
import sys

for _p in ("/opt/trn_rl_repo", "/root/.axon_site/_ro/trn_rl_repo"):
    if _p not in sys.path:
        sys.path.append(_p)

import numpy as np
import ml_dtypes

BF16 = ml_dtypes.bfloat16
F16 = np.float16
F8 = ml_dtypes.float8_e4m3

B, I, O, M = 128, 2048, 2048, 20
NCORES = 8
OC = O // NCORES          # 256 output cols per core
KC = I // 128             # 16 contraction chunks

MODE = os.environ.get("ACCURACY_MODE", "f16f8")

_CACHE: dict = {}
LAST_EXEC_TIME_NS = None


def _install_ntff_shim():
    """antenv.axon_hooks is absent in this image; recreate it so the
    trace=True path of run_bass_kernel_spmd can reach NTFF profiling."""
    import types
    import antenv

    if getattr(antenv, "axon_hooks", None) is not None:
        return
    hooks_mod = types.ModuleType("antenv.axon_hooks")
    _hook = [None]
    hooks_mod.set_axon_ntff_profile_hook = lambda h: _hook.__setitem__(0, h)
    hooks_mod.get_axon_ntff_profile_hook = lambda: _hook[0]
    sys.modules["antenv.axon_hooks"] = hooks_mod
    antenv.axon_hooks = hooks_mod
    try:
        from trn_agent_boot.trn_boot import _ntff_profile_via_ctypes

        hooks_mod.set_axon_ntff_profile_hook(
            _ntff_profile_via_ctypes("/opt/axon/libaxon_pjrt.so")
        )
    except Exception:
        pass
    import concourse.bass_utils as bass_utils

    bass_utils.upload_artifacts = lambda tmpdir: "local://" + tmpdir


def _build(counts: tuple, mode: str):
    import concourse.bass as bass
    import concourse.tile as tile
    from concourse import bacc, mybir

    offs = np.concatenate([[0], np.cumsum(counts)]).astype(int)

    nc = bacc.Bacc("TRN2", target_bir_lowering=False, debug=False, num_devices=NCORES)
    bf = mybir.dt.bfloat16
    f16 = mybir.dt.float16
    f8 = mybir.dt.float8e4
    f32 = mybir.dt.float32

    if mode == "f16f8":
        dt_a, dt_b, dt_x12 = f16, f8, f16
        comb = 2.0 ** -9
    else:  # bf16x2: T3 = Wlo @ xhi at scale 1
        dt_a, dt_b, dt_x12 = bf, bf, bf
        comb = 1.0

    # f16f8: T3 runs as fp8 DoubleRow (2 fp8 weights/PE cell, 0.5 cyc/row):
    # contraction chunks of 256 as [p, pair] with i = k'*256 + pair*128 + p.
    dr = mode == "f16f8"
    KD = KC // 2

    wa_d = nc.dram_tensor("wa", [M, 128, KC * OC], dt_a, kind="ExternalInput").ap()
    if dr:
        wb_d = nc.dram_tensor("wb", [M, 128, KD, 2, OC], dt_b, kind="ExternalInput").ap()
        x3_d = nc.dram_tensor("x3", [128, KD, 2, 128], dt_b, kind="ExternalInput").ap()
    else:
        wb_d = nc.dram_tensor("wb", [M, 128, KC * OC], dt_b, kind="ExternalInput").ap()
        x3_d = nc.dram_tensor("x3", [128, KC * 128], dt_b, kind="ExternalInput").ap()
    x1_d = nc.dram_tensor("x1", [128, KC * 128], dt_x12, kind="ExternalInput").ap()
    x2_d = nc.dram_tensor("x2", [128, KC * 128], dt_x12, kind="ExternalInput").ap()
    bh_d = nc.dram_tensor("bh", [2, M * OC], bf, kind="ExternalInput").ap()
    out_d = nc.dram_tensor("out", [B, OC], f32, kind="ExternalOutput").ap()

    with tile.TileContext(nc) as tc:
        with (
            tc.tile_pool(name="w", bufs=6) as wpool,
            tc.tile_pool(name="x", bufs=1) as xpool,
            tc.tile_pool(name="consts", bufs=1) as cpool,
            tc.tile_pool(name="o", bufs=3) as opool,
            tc.tile_pool(name="ps", bufs=4, space=bass.MemorySpace.PSUM) as pspool,
        ):
            # critical-path-first: x1 + mode 0's main weights go on the sync
            # ring; everything else rides the scalar HWDGE ring in parallel.
            x1t = xpool.tile([128, KC * 128], dt_x12, tag="x1")
            nc.sync.dma_start(x1t[:], x1_d[:])
            x2t = xpool.tile([128, KC * 128], dt_x12, tag="x2")
            nc.scalar.dma_start(x2t[:], x2_d[:])
            if dr:
                x3t = xpool.tile([128, KD, 2, 128], dt_b, tag="x3")
            else:
                x3t = xpool.tile([128, KC * 128], dt_b, tag="x3")
            nc.scalar.dma_start(x3t[:], x3_d[:])
            bt = cpool.tile([2, M * OC], bf)
            nc.scalar.dma_start(bt[:], bh_d[:])
            ones = cpool.tile([2, 128], bf)
            nc.vector.memset(ones[:], 1.0)

            for m in range(M):
                cm = int(counts[m])
                if cm == 0:
                    continue
                o0 = int(offs[m])
                # two half-tiles so the first T1 matmuls start after 0.5 MB
                wa = wpool.tile([128, KC * OC], dt_a, tag="wa")
                half = KC * OC // 2
                nc.sync.dma_start(wa[:, 0:half], wa_d[m, :, 0:half])
                nc.sync.dma_start(wa[:, half:], wa_d[m, :, half:])
                if dr:
                    wb = wpool.tile([128, KD, 2, OC], dt_b, tag="wb")
                else:
                    wb = wpool.tile([128, KC * OC], dt_b, tag="wb")
                nc.sync.dma_start(wb[:], wb_d[m])

                ps_main = pspool.tile([128, OC], f32, tag="ps_main")
                ps23 = pspool.tile([128, OC], f32, tag="ps23")

                # T1: W1 @ x1 -> ps_main
                for k in range(KC):
                    nc.tensor.matmul(
                        ps_main[0:cm, :],
                        x1t[:, k * 128 + o0 : k * 128 + o0 + cm],
                        wa[:, k * OC : (k + 1) * OC],
                        start=(k == 0),
                        stop=False,
                    )
                # bias (scaled by 64 on host): ones[2,cm].T @ [bh; bl]
                nc.tensor.matmul(
                    ps_main[0:cm, :],
                    ones[:, 0:cm],
                    bt[:, m * OC : (m + 1) * OC],
                    start=False,
                    stop=True,
                )
                # T2: W1 @ x2s, T3: W2 @ x3 -> ps23 (both at 2^15 scale)
                for k in range(KC):
                    nc.tensor.matmul(
                        ps23[0:cm, :],
                        x2t[:, k * 128 + o0 : k * 128 + o0 + cm],
                        wa[:, k * OC : (k + 1) * OC],
                        start=(k == 0),
                        stop=False,
                    )
                if dr:
                    for kp in range(KD):
                        nc.tensor.matmul(
                            ps23[0:cm, :],
                            x3t[:, kp, :, o0 : o0 + cm],
                            wb[:, kp, :, :],
                            start=False,
                            stop=(kp == KD - 1),
                            perf_mode=mybir.MatmulPerfMode.DoubleRow,
                        )
                else:
                    for k in range(KC):
                        nc.tensor.matmul(
                            ps23[0:cm, :],
                            x3t[:, k * 128 + o0 : k * 128 + o0 + cm],
                            wb[:, k * OC : (k + 1) * OC],
                            start=False,
                            stop=(k == KC - 1),
                        )

                tmp = opool.tile([128, OC], f32, tag="tmp")
                nc.vector.tensor_scalar_mul(tmp[0:cm, :], ps23[0:cm, :], comb)
                ot = opool.tile([128, OC], f32, tag="ot")
                nc.vector.tensor_add(ot[0:cm, :], ps_main[0:cm, :], tmp[0:cm, :])
                nc.scalar.dma_start(out_d[o0 : o0 + cm, :], ot[0:cm, :])

    nc.compile()
    return nc


def _w_layout(plane: np.ndarray, dt) -> np.ndarray:
    """[m, o, i] -> [core, m, p, k*OC] so each (mode, core) DMA is one
    contiguous-per-partition [128, KC*OC] tile with rhs chunks in order."""
    return np.ascontiguousarray(
        plane.reshape(M, NCORES, OC, KC, 128).transpose(1, 0, 4, 3, 2).astype(dt)
    ).reshape(NCORES, M, 128, KC * OC)


def _x_layout(plane: np.ndarray, dt) -> np.ndarray:
    """[s, i] -> [p, k*128] (lhsT chunks: partition = i within chunk)."""
    return np.ascontiguousarray(
        plane.reshape(B, KC, 128).transpose(2, 1, 0).astype(dt)
    ).reshape(128, KC * 128)


def _w_layout_dr(plane: np.ndarray, dt) -> np.ndarray:
    """[m, o, i] -> [core, m, p, k', pair, cc] for fp8 DoubleRow rhs tiles
    (i = k'*256 + pair*128 + p)."""
    return np.ascontiguousarray(
        plane.reshape(M, NCORES, OC, KC // 2, 2, 128)
        .transpose(1, 0, 5, 3, 4, 2)
        .astype(dt)
    )


def _x_layout_dr(plane: np.ndarray, dt) -> np.ndarray:
    """[s, i] -> [p, k', pair, s] for fp8 DoubleRow lhsT tiles."""
    return np.ascontiguousarray(
        plane.reshape(B, KC // 2, 2, 128).transpose(3, 1, 2, 0).astype(dt)
    )


def kernel(x, weights, biases, mode_idx):
    global LAST_EXEC_TIME_NS

    x = np.asarray(x, dtype=np.float32)
    weights = np.asarray(weights, dtype=np.float32)
    biases = np.asarray(biases, dtype=np.float32)
    mode_idx_np = np.asarray(mode_idx).astype(np.int64)

    assert x.shape == (B, I) and weights.shape == (M, O, I)
    assert biases.shape == (M, O) and mode_idx_np.shape == (B,)

    order = np.argsort(mode_idx_np, kind="stable")
    counts = np.bincount(mode_idx_np, minlength=M)
    key = (tuple(int(c) for c in counts), MODE)

    if key not in _CACHE:
        _CACHE[key] = _build(key[0], MODE)
    nc = _CACHE[key]

    xs = x[order]                                    # [B, I] sorted by mode

    if MODE == "f16f8":
        ws = weights * np.float32(64.0)
        w1 = ws.astype(F16)
        r = ws - w1.astype(np.float32)
        WA = _w_layout(w1, F16)
        WB = _w_layout_dr(r * np.float32(512.0), F8)
        del ws, r

        x1 = xs.astype(F16)
        x2 = (xs - x1.astype(np.float32)) * np.float32(512.0)
        X1 = _x_layout(x1, F16)
        X2 = _x_layout(x2, F16)
        X3 = _x_layout_dr(xs, F8)

        bs = biases * np.float32(64.0)
        out_scale = np.float32(1.0 / 64.0)
    else:  # bf16x2
        w1 = weights.astype(BF16)
        r = weights - w1.astype(np.float32)
        WA = _w_layout(w1, BF16)
        WB = _w_layout(r, BF16)

        x1 = xs.astype(BF16)
        x2 = xs - x1.astype(np.float32)
        X1 = _x_layout(x1, BF16)
        X2 = _x_layout(x2, BF16)
        X3 = X1                                      # T3 = Wlo @ xhi

        bs = biases
        out_scale = np.float32(1.0)

    bh = bs.astype(BF16)
    bl = (bs - bh.astype(np.float32)).astype(BF16)
    bpl = np.stack([bh, bl], 0)                      # [t, m, o]
    BH = np.ascontiguousarray(
        bpl.reshape(2, M, NCORES, OC).transpose(2, 0, 1, 3)
    ).reshape(NCORES, 2, M * OC)

    in_maps = [
        {"wa": WA[c], "wb": WB[c], "x1": X1, "x2": X2, "x3": X3, "bh": BH[c]}
        for c in range(NCORES)
    ]

    from concourse.bass_utils import run_bass_kernel_spmd

    trace = bool(int(os.environ.get("BASS_KERNEL_TRACE", "0")))
    if trace:
        _install_ntff_shim()
    res = run_bass_kernel_spmd(
        nc,
        in_maps,
        list(range(NCORES)),
        trace=trace,
        trace_cores=list(range(NCORES)) if trace else None,
    )
    LAST_EXEC_TIME_NS = res.exec_time_ns

    sorted_out = np.concatenate(
        [res.results[c]["out"] for c in range(NCORES)], axis=1
    )                                                # [B, O] in sorted order
    out = np.empty((B, O), dtype=np.float32)
    out[order] = sorted_out * out_scale
    return out
